# revision 23
# baseline (speedup 1.0000x reference)
"""Trainium2 Bass kernel: multi-head attention block (B=4, N=2048, C=1024, H=16).

Sharding: 8 cores = (batch b in 0..3) x (head-group hg in 0..1, 8 heads each).
Each core computes qkv for its heads, full attention for its heads over its
batch, and a partial projection (its 512 rows of W_proj). Host sums the two
partials per batch and adds b_proj.

Device schedule (v2): the ScalarE exp stream (256 x [128,1024] activations,
~294us) is the critical path; everything else is emitted so the Tile
scheduler can pack it around the exp spine:
  - qkv projection, v-production and output-projection matmuls are thunk-ized
    and interleaved into the attention j-steps as PE filler.
  - DMA loads are chunk-interleaved (xT/wqk/wv per contraction chunk) so the
    first qk chains start ~2us into the kernel.
  - exp_pool is deep (20 tiles) so ACT can run ahead of the O matmuls while
    v-production catches up during the ramp.
  - O matmuls optionally row-split (contraction 64+64 in concurrent PE
    row-groups accumulating into the same PSUM bank) to halve O stream time.
  - proj quarters are emitted as soon as pair 3 finishes each q-block.

Layout choices (all matmuls bf16 inputs, fp32 PSUM accumulate):
  - q,k produced in transposed layout qkT[dim, token] so S^T = k^T-chunks.T @ q^T
    needs no on-chip transposes.
  - v produced in natural layout [token, 65*h] with a trailing ones column per
    head, so the O matmul lhsT=[v|ones] gives row 64 = softmax denominator and
    rows 0..63 = unnormalized o^T in one PSUM accumulation chain.
  - exp on ScalarE with fused scale=1/8; no max subtraction (logits bounded).
  - head PAIRS: the two heads' S matmuls sit at base partitions 0/64 (distinct
    PE row groups) and issue back-to-back, so they run concurrently.
"""

import os
import sys
from contextlib import ExitStack

import numpy as np
import ml_dtypes

import concourse.bass as bass
import concourse.tile as tile
from concourse import bacc, mybir
from concourse.bass import ds, ts
from concourse.bass_utils import run_bass_kernel_spmd

try:  # without the NTFF hook module, a stray BASS_TRACE=1 would crash the run
    from antenv.axon_hooks import get_axon_ntff_profile_hook  # noqa: F401
except ImportError:
    os.environ.setdefault("BASS_NEVER_TRACE", "1")

BF16 = mybir.dt.bfloat16
F32 = mybir.dt.float32
NP_BF16 = ml_dtypes.bfloat16

B, N, C = 4, 2048, 1024
H, D = 16, 64
HPC = 8            # heads per core
CD = HPC * D       # 512 local qkv dims per core
E = D + 1          # 65: 64 v dims + ones column

EXP_BUFS = int(os.environ.get("EXP_BUFS", "20"))

LAST_RESULTS = None  # stash for test harness (exec_time_ns, trace paths)


def _build_program(taps=False):
    nc = bacc.Bacc("TRN2", target_bir_lowering=False, debug=False)

    xT_d = nc.dram_tensor("xT", [C, N], BF16, kind="ExternalInput").ap()
    wqk_d = nc.dram_tensor("wqk", [C, 2 * CD], BF16, kind="ExternalInput").ap()
    wv_d = nc.dram_tensor("wv", [C, CD], BF16, kind="ExternalInput").ap()
    bqk_d = nc.dram_tensor("bqk", [128, 8], F32, kind="ExternalInput").ap()
    bv_d = nc.dram_tensor("bv", [1, CD], BF16, kind="ExternalInput").ap()
    wp_d = nc.dram_tensor("wp", [CD, C], BF16, kind="ExternalInput").ap()
    out_d = nc.dram_tensor("out", [N, C], BF16, kind="ExternalOutput").ap()
    if taps:
        tap_qkT = nc.dram_tensor("tap_qkT", [128, 8, N], BF16, kind="ExternalOutput").ap()
        tap_v = nc.dram_tensor("tap_v", [128, 16, HPC * E], BF16, kind="ExternalOutput").ap()
        tap_ex = nc.dram_tensor("tap_ex", [128, 1024], BF16, kind="ExternalOutput").ap()
        tap_ot = nc.dram_tensor("tap_ot", [65, 512], F32, kind="ExternalOutput").ap()
        tap_recb = nc.dram_tensor("tap_recb", [64, 512], F32, kind="ExternalOutput").ap()
        tap_oT = nc.dram_tensor("tap_oT", [128, 4, N], BF16, kind="ExternalOutput").ap()

    with tile.TileContext(nc) as tc, ExitStack() as ctx:
        singles = ctx.enter_context(tc.tile_pool(name="singles", bufs=1))
        ps_pool = ctx.enter_context(tc.tile_pool(name="ps", bufs=2, space="PSUM"))
        st_pool = ctx.enter_context(tc.tile_pool(name="stp", bufs=2, space="PSUM"))
        ot_pool = ctx.enter_context(tc.tile_pool(name="ot", bufs=2, space="PSUM"))
        exp_pool = ctx.enter_context(tc.tile_pool(name="expp", bufs=EXP_BUFS))
        misc = ctx.enter_context(tc.tile_pool(name="misc", bufs=4))
        ob_pool = ctx.enter_context(tc.tile_pool(name="ob", bufs=4))

        # Persistent SBUF tensors, chunk-major: [partition, chunk, free].
        xT_sb = singles.tile([128, 8, N], BF16)        # x^T   [c, token]
        wqk_sb = singles.tile([128, 8, 2 * CD], BF16)  # W_qk  [c, m]
        wv_sb = singles.tile([128, 8, CD], BF16)       # W_v   [c, n]
        bqk_sb = singles.tile([128, 8], F32)
        bv_sb = singles.tile([1, CD], BF16)
        wp_sb = singles.tile([128, 4, C], BF16)        # W_proj [hd, n]
        qkT_sb = singles.tile([128, 8, N], BF16)       # chunks 0..3 = q, 4..7 = k
        v_sb = singles.tile([128, 16, HPC * E], BF16)  # [token-in-chunk, tchunk, h*(64+1)]
        oT_sb = singles.tile([128, 4, N], BF16)        # o^T, proj lhsT layout

        # DMA order: qk's inputs first (wqk_kc + xT_kc pairs) so the first S
        # steps and exp start ~17us in; wv after (O lags behind exp anyway).
        nc.sync.dma_start(bqk_sb, bqk_d)
        nc.sync.dma_start(bv_sb, bv_d)
        for kc in range(8):
            nc.sync.dma_start(wqk_sb[:, kc], wqk_d.rearrange("(c p) m -> p c m", p=128)[:, kc])
            nc.sync.dma_start(xT_sb[:, kc], xT_d.rearrange("(c p) t -> p c t", p=128)[:, kc])
        for kc in range(8):
            nc.sync.dma_start(wv_sb[:, kc], wv_d.rearrange("(c p) m -> p c m", p=128)[:, kc])
        # ones columns of v (col 64 of each head), all 16 chunks in one memset
        nc.vector.memset(
            v_sb.rearrange("p t (h e) -> p t h e", e=E)[:, :, :, D : D + 1], 1.0
        )
        # bv broadcast to all 128 token partitions (fin folds it into the
        # PSUM->SBUF copy, replacing the per-chunk ones x bv matmul)
        bv_b = singles.tile([128, CD], BF16)
        nc.gpsimd.partition_broadcast(bv_b, bv_sb)
        nc.sync.dma_start(wp_sb, wp_d.rearrange("(c p) n -> p c n", p=128))

        # v chunk t: natural layout + bias via ones x bv matmul; 10 thunks.
        def v_thunks(t):
            hold = {}
            thunks = []

            def mk(kc, t=t, hold=hold):
                def f():
                    if kc == 0:
                        hold["ps"] = ps_pool.tile([128, 512], F32, tag="big", name="vps")
                    nc.tensor.matmul(
                        hold["ps"],
                        xT_sb[:, kc, ts(t, 128)],
                        wv_sb[:, kc, :],
                        start=(kc == 0),
                        stop=(kc == 7),
                    )
                return f

            def fin(t=t, hold=hold):
                vv = v_sb[:, t].rearrange("p (h e) -> p h e", e=E)
                nc.vector.tensor_add(
                    vv[:, :, 0:D],
                    hold["ps"].rearrange("p (h d) -> p h d", d=D),
                    bv_b.rearrange("p (h d) -> p h d", d=D),
                )

            for kc in range(8):
                thunks.append(mk(kc))
            thunks.append(fin)
            return thunks

        # qk chain for (chunk m, token block i4q): 8 matmuls + bias add.
        def qk_chain(m, i4q):
            ps = ps_pool.tile([128, 512], F32, tag="big", name="qkps")
            for kc in range(8):
                nc.tensor.matmul(
                    ps,
                    wqk_sb[:, kc, ts(m, 128)],
                    xT_sb[:, kc, ds(i4q * 512, 512)],
                    start=(kc == 0),
                    stop=(kc == 7),
                )
            nc.vector.tensor_scalar_add(
                qkT_sb[:, m, ds(i4q * 512, 512)], ps, bqk_sb[:, ds(m, 1)]
            )

        def emit_proj(t):
            pp0 = ps_pool.tile([128, 512], F32, tag="big")
            pp1 = ps_pool.tile([128, 512], F32, tag="big")
            for hc in range(4):
                nc.tensor.matmul(
                    pp0, oT_sb[:, hc, ts(t, 128)], wp_sb[:, hc, ds(0, 512)],
                    start=(hc == 0), stop=(hc == 3),
                )
                nc.tensor.matmul(
                    pp1, oT_sb[:, hc, ts(t, 128)], wp_sb[:, hc, ds(512, 512)],
                    start=(hc == 0), stop=(hc == 3),
                )
            for nh, pp in ((0, pp0), (1, pp1)):
                ob = ob_pool.tile([128, 512], BF16)
                nc.vector.tensor_copy(ob, pp)
                nc.sync.dma_start(out_d[ts(t, 128), ds(nh * 512, 512)], ob)

        # ---- filler scheduling. NOTE: emission order IS program order — a
        # consumer emitted before its producer reads stale data (no implicit
        # dep). All fillers are emitted at NORMAL priority but spread across
        # the attention steps (sched maps (p, i4, j) -> thunks) so no step's
        # PE load exceeds the ACT budget for long: v chunk j+1 lands in
        # (p0, i4=0, j); qk chains for later pairs spread ~1 per 4 steps.
        sched = {}

        def at_step(p, i4, j, th):
            sched.setdefault((p, i4, j), []).append(th)

        def qk_thunk(m, i4q):
            return lambda: qk_chain(m, i4q)

        # Pre-attention critical path: only what S (pair 0, i4=0, j=0) needs.
        qk_chain(0, 0)
        qk_chain(4, 0)
        for th in v_thunks(0):
            th()
        # v chunk j+1 just before the step whose O consumes it
        for j in range(15):
            for th in v_thunks(j + 1):
                at_step(0, 0, j, th)
        # k chains for pair 0 just ahead of the j-blocks that read them
        at_step(0, 0, 0, qk_thunk(4, 1))
        at_step(0, 0, 4, qk_thunk(4, 2))
        at_step(0, 0, 8, qk_thunk(4, 3))
        # q chains for pair 0's later i4 blocks
        at_step(0, 0, 12, qk_thunk(0, 1))
        at_step(0, 1, 0, qk_thunk(0, 2))
        at_step(0, 1, 8, qk_thunk(0, 3))
        # later pairs' chains: one chain per 4 steps
        spread = [(1, [(0, 1, 12), (0, 2, 0), (0, 2, 4), (0, 2, 8)]),
                  (5, [(0, 2, 12), (0, 3, 0), (0, 3, 4), (0, 3, 8)]),
                  (2, [(1, 0, 0), (1, 0, 4), (1, 0, 8), (1, 0, 12)]),
                  (6, [(1, 1, 0), (1, 1, 4), (1, 1, 8), (1, 1, 12)]),
                  (3, [(1, 2, 0), (1, 2, 4), (1, 2, 8), (1, 2, 12)]),
                  (7, [(1, 3, 0), (1, 3, 4), (1, 3, 8), (1, 3, 12)])]
        for m, slots in spread:
            for i4q, (sp, si, sj) in enumerate(slots):
                at_step(sp, si, sj, qk_thunk(m, i4q))

        # Phase 2: per head PAIR. The two heads' S matmuls target different PE
        # row groups (base partitions 0 / 64) so back-to-back issue runs them
        # concurrently; both write one [128, 1024] st tile and share one exp.
        for p in range(4):
            qA, qB = qkT_sb[0:64, p], qkT_sb[64:128, p]
            kA, kB = qkT_sb[0:64, 4 + p], qkT_sb[64:128, 4 + p]
            hA, hB = 2 * p, 2 * p + 1
            for i4 in range(4):
                i0 = i4 * 512
                otA = ot_pool.tile([65, 512], F32, tag="ot")
                otB = ot_pool.tile([65, 512], F32, tag="ot")
                for j in range(16):
                    for th in sched.get((p, i4, j), ()):
                        th()
                    # pair 3: previous q-block's proj quarters, 1 per 4 steps
                    if p == 3 and i4 > 0 and j % 4 == 0:
                        emit_proj(4 * (i4 - 1) + j // 4)
                    st = st_pool.tile([128, 1024], F32, tag="st")
                    nc.tensor.matmul(
                        st[:, 0:512], kA[:, ts(j, 128)], qA[:, ds(i0, 512)],
                        start=True, stop=True,
                    )
                    nc.tensor.matmul(
                        st[:, 512:1024], kB[:, ts(j, 128)], qB[:, ds(i0, 512)],
                        start=True, stop=True,
                    )
                    ex = exp_pool.tile([128, 1024], BF16)
                    nc.scalar.activation(
                        ex, st, mybir.ActivationFunctionType.Exp, scale=float(D) ** -0.5
                    )
                    if taps and p == 0 and i4 == 0 and j == 0:
                        nc.sync.dma_start(tap_ex, ex)
                    vvj = v_sb[:, j].rearrange("p (h e) -> p h e", e=E)
                    nc.tensor.matmul(
                        otA, vvj[:, hA], ex[:, 0:512], start=(j == 0), stop=(j == 15)
                    )
                    nc.tensor.matmul(
                        otB, vvj[:, hB], ex[:, 512:1024], start=(j == 0), stop=(j == 15)
                    )
                for hp_, ot in ((0, otA), (64, otB)):
                    # Copy the whole accumulator out first: frees the PSUM slot
                    # fast; same DVE cost as one row (partitions are parallel).
                    otc = misc.tile([65, 512], F32, tag="otc")
                    nc.vector.tensor_copy(otc, ot)
                    if taps and p == 0 and i4 == 0 and hp_ == 0:
                        nc.sync.dma_start(tap_ot, otc)
                    # Softmax denominators: lane-scatter so reciprocal runs on
                    # 128 lanes x 4 elems instead of 1 lane x 512 (DVE divide
                    # is ~8 cycles/elem serial per lane).
                    s_t = misc.tile([128, 4], F32, tag="sct")
                    nc.sync.dma_start(s_t, otc[64:65])
                    r_t = misc.tile([128, 4], F32, tag="rct")
                    nc.vector.reciprocal(r_t, s_t)
                    rec0 = misc.tile([1, 512], F32, tag="rec0")
                    nc.sync.dma_start(rec0, r_t)
                    recb = misc.tile([64, 512], F32, tag="recb")
                    nc.gpsimd.partition_broadcast(recb, rec0)
                    if taps and p == 0 and i4 == 0 and hp_ == 0:
                        nc.sync.dma_start(tap_recb, recb)
                    if hp_ == 0:
                        # same partitions: write oT_sb directly from DVE
                        nc.vector.tensor_mul(
                            oT_sb[0:64, p, ds(i0, 512)], otc[0:64], recb
                        )
                    else:
                        tmp = misc.tile([64, 512], BF16, tag="tmp")
                        nc.vector.tensor_mul(tmp, otc[0:64], recb)
                        nc.sync.dma_start(oT_sb[64:128, p, ds(i0, 512)], tmp)
        # the last q-block's proj quarters (its oT only just completed)
        for t in range(12, 16):
            emit_proj(t)

        if taps:
            nc.sync.dma_start(tap_qkT, qkT_sb)
            nc.sync.dma_start(tap_v, v_sb)
            nc.sync.dma_start(tap_oT, oT_sb)

    nc.compile()
    return nc


_PROGRAM = None


def kernel(x, W_qkv, b_qkv, W_proj, b_proj):
    global _PROGRAM, LAST_RESULTS
    x = np.asarray(x, dtype=np.float32)
    W_qkv = np.asarray(W_qkv, dtype=np.float32)
    b_qkv = np.asarray(b_qkv, dtype=np.float32)
    W_proj = np.asarray(W_proj, dtype=np.float32)
    b_proj = np.asarray(b_proj, dtype=np.float32)

    if _PROGRAM is None:
        _PROGRAM = _build_program()
    nc = _PROGRAM

    in_maps = []
    for core in range(8):
        b, hg = core // 2, core % 2
        h0 = hg * HPC
        sl = slice(h0 * D, h0 * D + CD)
        wq = W_qkv[:, 0 * C :][:, sl]
        wk = W_qkv[:, 1 * C :][:, sl]
        wv = W_qkv[:, 2 * C :][:, sl]
        bq = b_qkv[0 * C :][sl]
        bk = b_qkv[1 * C :][sl]
        bv = b_qkv[2 * C :][sl]
        in_maps.append(
            {
                "xT": np.ascontiguousarray(x[b].T).astype(NP_BF16),
                "wqk": np.concatenate([wq, wk], axis=1).astype(NP_BF16),
                "wv": np.ascontiguousarray(wv).astype(NP_BF16),
                "bqk": np.concatenate([bq, bk]).reshape(8, 128).T.astype(np.float32).copy(),
                "bv": bv.reshape(1, CD).astype(NP_BF16),
                "wp": np.ascontiguousarray(W_proj[sl, :]).astype(NP_BF16),
            }
        )

    res = run_bass_kernel_spmd(nc, in_maps, list(range(8)))
    LAST_RESULTS = res
    out = np.empty((B, N, C), dtype=np.float32)
    for b in range(B):
        out[b] = (
            res.results[2 * b]["out"].astype(np.float32)
            + res.results[2 * b + 1]["out"].astype(np.float32)
            + b_proj[None, :]
        )
    return out
